# revision 6
# baseline (speedup 1.0000x reference)
"""BinaryLinear forward on 8 Trainium2 NeuronCores.

Computes out = x @ sign(weight).T for x:[16384,2048] (values in {-1,+1}),
weight:[2048,2048] -> out:[16384,2048] fp32 — bit-exact vs the fp32
reference.

Strategy (data-parallel per the sharding hint): shard x rows across the 8
cores (2048 each), replicate the binarized weight. Both operands are
exactly +/-1, so they are cast to fp8e4 (exact) and the matmul runs in
DoubleRow perf mode (2 fp8 weights per PE cell -> K=256 per matmul, 2x
bf16 throughput) accumulating in fp32 PSUM; sums are even integers
<= 2048, exact in fp32 and also in the fp16 used for the output DMA
(halved write traffic), upcast to fp32 on the host.

Kernel layout/scheduling notes:
 - x and w are pre-transposed on the host so K lands on the SBUF
   partition dim with unit-stride DMAs; both stay SBUF-resident
   (4.2 MB each per core).
 - input chunks alternate between the sync and scalar HWDGE queues
   (~150 GB/s each) in exactly the order compute consumes them, so the
   PE starts ~10.5us in and never starves thereafter.
 - dummy warmup matmuls fill the initial DMA wait so the PE HAM clock
   gate is already 8/8 (2.4 GHz) when real work arrives.
 - steady-state outputs ride the gpsimd SWDGE queue; the final phase's
   outputs use the by-then-idle HWDGE queues to keep the exit drain
   short.
Measured: ~130 us HW exec per NEFF (PE stream within ~2% of the fp8
DoubleRow silicon floor of 108.4 us for 512 matmuls).
"""

import numpy as np
import ml_dtypes

import concourse.mybir as mybir
import concourse.tile as tile
from concourse import bacc
from concourse.bass_utils import run_bass_kernel_spmd

M, K, O = 16384, 2048, 2048
N_CORES = 8
MS = M // N_CORES
P = 128
KO2 = K // (2 * P)         # 8 double-row k-chunks
NB = 512
NJ = O // NB               # 4 o-blocks
MO = MS // P               # 16 m-blocks
MH = 4                     # m-blocks per phase
MB = MH * P                # 512 m-cols per phase
NMH = MO // MH             # 4 m-phases per o-block

FP8 = mybir.dt.float8e4

_CACHE = {}


def _build():
    if "nc" in _CACHE:
        return _CACHE["nc"]

    nc = bacc.Bacc("TRN2", target_bir_lowering=False, debug=False,
                   num_devices=N_CORES)
    xT = nc.dram_tensor("xT", [K, MS], FP8, kind="ExternalInput")
    wT = nc.dram_tensor("wT", [K, O], FP8, kind="ExternalInput")
    out = nc.dram_tensor("out", [MS, O], mybir.dt.float16,
                         kind="ExternalOutput")

    xT_v = xT.ap().rearrange("(kc ks pi) m -> pi kc ks m", pi=P, ks=2)
    wT_v = wT.ap().rearrange("(kc ks pi) o -> pi kc ks o", pi=P, ks=2)
    out_v = out.ap().rearrange("(mo pi) o -> pi mo o", pi=P)

    with tile.TileContext(nc) as tc:
        with tc.tile_pool(name="xres", bufs=1) as x_pool, \
             tc.tile_pool(name="wres", bufs=1) as w_pool, \
             tc.tile_pool(name="outs", bufs=8) as out_pool, \
             tc.tile_pool(name="psum", bufs=8, space="PSUM") as psum_pool:

            x_t = [[None] * NMH for _ in range(KO2)]
            w_t = [[None] * NJ for _ in range(KO2)]
            alt = [0]

            def _eng():
                alt[0] += 1
                return nc.sync if alt[0] % 2 == 0 else nc.scalar

            def load_x(kc, mq):
                t = x_pool.tile([P, 2, MB], FP8, tag=f"x{kc}_{mq}",
                                name=f"x{kc}_{mq}")
                _eng().dma_start(t[:], xT_v[:, kc, :, mq * MB:(mq + 1) * MB])
                x_t[kc][mq] = t

            def load_w(kc, j):
                t = w_pool.tile([P, 2, NB], FP8, tag=f"w{kc}_{j}",
                                name=f"w{kc}_{j}")
                _eng().dma_start(t[:], wT_v[:, kc, :, j * NB:(j + 1) * NB])
                w_t[kc][j] = t

            # PE warmup: dummy matmuls on a zeroed tile fill the DMA-wait
            # window so the HAM clock gate is already 8/8 (2.4 GHz) when the
            # first real matmul issues. Sized to finish just before the first
            # input chunk lands (~10.5us); the <3.4us idle gap after them is
            # too short for the HAM to re-throttle.
            zw = w_pool.tile([P, 2, NB], FP8, tag="zwarm", name="zwarm")
            nc.vector.memset(zw[:], 0)
            pz = psum_pool.tile([P, NB], mybir.dt.float32, tag="ps",
                                name="ps_warm")
            for _ in range(10):
                nc.tensor.matmul(
                    pz[:], zw[:, :, 0:P], zw[:],
                    start=True, stop=True,
                    perf_mode=mybir.MatmulPerfMode.DoubleRow,
                )

            # Emission (= per-queue arrival) order mirrors consumption order.
            for kc in range(KO2):
                load_x(kc, 0)
                load_w(kc, 0)
            for mq in range(1, NMH):
                for kc in range(KO2):
                    load_x(kc, mq)
            for j in range(1, NJ):
                for kc in range(KO2):
                    load_w(kc, j)

            # Final two phase-groups are half-size so the end-of-kernel cast+
            # DMA tail is shorter (fewer serialized PSUM evictions after the
            # very last matmul).
            full = [(s, MH) for s in range(0, MO, MH)]
            tail_split = full[:-1] + [(MO - MH, 2), (MO - 2, 2)]
            for j in range(NJ):
                groups = tail_split if j == NJ - 1 else full
                for gi, (mo0, gsz) in enumerate(groups):
                    psums = [psum_pool.tile([P, NB], mybir.dt.float32,
                                            tag="ps", name=f"ps_{j}_{gi}_{i}")
                             for i in range(gsz)]
                    for kc in range(KO2):
                        for mi in range(gsz):
                            mo = mo0 + mi
                            mh, mr = divmod(mo, MH)
                            nc.tensor.matmul(
                                psums[mi][:],
                                x_t[kc][mh][:, :, mr * P:(mr + 1) * P],
                                w_t[kc][j][:],
                                start=(kc == 0),
                                stop=(kc == KO2 - 1),
                                perf_mode=mybir.MatmulPerfMode.DoubleRow,
                            )
                    last_phase = (j == NJ - 1 and gi == len(groups) - 1)
                    for mi in range(gsz):
                        mo = mo0 + mi
                        ot = out_pool.tile([P, NB], mybir.dt.float16,
                                           tag="ot", name=f"ot_{j}_{gi}_{mi}")
                        nc.vector.tensor_copy(out=ot[:], in_=psums[mi][:])
                        # Outputs ride the (slow) gpsimd SWDGE queue, which
                        # keeps pace; the final phase uses the by-now-idle
                        # HWDGE queues so the exit drain isn't waiting on the
                        # SWDGE backlog. (Putting *all* outputs there blocks
                        # late input triggers behind cast-waits — measured.)
                        if last_phase:
                            oeng = nc.sync if mi % 2 == 0 else nc.scalar
                        else:
                            oeng = nc.gpsimd
                        oeng.dma_start(
                            out_v[:, mo, j * NB:(j + 1) * NB], ot[:])

    nc.compile()
    _CACHE["nc"] = nc
    return nc


def prepare_in_maps(x, weight):
    x = np.asarray(x, dtype=np.float32)
    weight = np.asarray(weight, dtype=np.float32)
    # sign(sign(w) + 0.5): maps 0 -> +1, else +/-1 (matches the reference)
    bw = np.sign(np.sign(weight, dtype=np.float32) + np.float32(0.5))
    wT_h = np.ascontiguousarray(bw.T.astype(ml_dtypes.float8_e4m3))
    xT_h = np.ascontiguousarray(x.T.astype(ml_dtypes.float8_e4m3))
    return [
        {"xT": np.ascontiguousarray(xT_h[:, c * MS:(c + 1) * MS]), "wT": wT_h}
        for c in range(N_CORES)
    ]


def gather_output(results):
    return np.concatenate(
        [results[c]["out"] for c in range(N_CORES)], axis=0
    ).astype(np.float32)


def kernel(x, weight):
    nc = _build()
    in_maps = prepare_in_maps(x, weight)
    res = run_bass_kernel_spmd(nc, in_maps, core_ids=list(range(N_CORES)))
    return gather_output(res.results)
